# revision 1
# baseline (speedup 1.0000x reference)
"""TRN2 Bass kernel for nn_GQA_22436909154699 — optimized v3.

Reference math: softmax over a size-1 axis is identically 1.0, so
    out[b,l,g,h,:] = v[b,l,g,:]          (v = v-half of x @ Wkv + bkv)
The q projection (x @ Wq) never affects the output.  The kernel computes
    y = x @ Wv + bv                      (K=2048, N=256)
data-parallel over tokens across 8 NeuronCores (2048 tokens each).

Key optimizations vs the fp32 baseline:
  - bf16 operands (fp32 PSUM accumulate): fp32 moving operands stream
    the PE at 1/4 rate, bf16 at full rate; also halves x DMA traffic.
  - weight-stationary matmuls: lhsT = Wv k-tile [128k x 128n], rhs =
    x [128k x 256t] -> psum [128n x 256t].
  - the device emits only the 256 unique v-columns per token; the 8x
    heads-per-group replication is pure data movement done at unshard.
  - HWDGE DMAs on one queue complete FIFO in issue order, so the issue
    order is delivery order: wv half 0, first x block, wv half 1, bias,
    then remaining x blocks.  256-token x blocks (1 MB) keep the PE
    chasing the DMA stream with minimal first-block latency.
  - dummy warm-up matmuls on zeros run during the DMA fill so the HAM
    clock gate is at 8/8 (2.4 GHz) when real matmuls start.
"""

import os

import numpy as np

# Problem constants (hardcoded; harness runs kernel.py standalone).
B, L, E = 4, 4096, 2048
G, HPG, D = 4, 8, 64
NV = G * D  # 256 v-columns
NH = NV // 128  # 2 column halves (PE stationary is 128 wide)
NCORES = 8
TOK = B * L  # 16384 tokens
TPC = TOK // NCORES  # 2048 tokens per core
TBLK = 256  # tokens per matmul rhs
TB = TPC // TBLK  # 8 token blocks per core
KO = E // 128  # 16 contraction tiles

_CACHE: dict = {}
LAST_RESULTS = None


def _build(warmup: int):
    import concourse.bacc as bacc
    import concourse.mybir as mybir
    import concourse.tile as tile

    F32 = mybir.dt.float32
    BF16 = mybir.dt.bfloat16

    nc = bacc.Bacc(
        "TRN2", target_bir_lowering=False, debug=False, num_devices=NCORES
    )
    xt_d = nc.dram_tensor("xt", [TB, 128, KO, TBLK], BF16, kind="ExternalInput")
    wv_d = nc.dram_tensor("wv", [NH, 128, KO, 128], BF16, kind="ExternalInput")
    bias_d = nc.dram_tensor("bias", [128, NH], F32, kind="ExternalInput")
    out_d = nc.dram_tensor("out", [NH, TB, 128, TBLK], F32, kind="ExternalOutput")

    with tile.TileContext(nc) as tc:
        with (
            tc.tile_pool(name="const", bufs=1) as cpool,
            tc.tile_pool(name="xin", bufs=TB) as xpool,
            tc.tile_pool(name="obuf", bufs=4) as opool,
            tc.tile_pool(name="ps", bufs=8, space="PSUM") as ppool,
        ):
            # PE warm-up on zeros: keeps the HAM activity window busy
            # while the first DMAs land, so real matmuls start at 2.4 GHz.
            if warmup:
                zt = cpool.tile([128, 512], BF16)
                nc.vector.memset(zt[:], 0.0)
                wps = ppool.tile([128, 512], F32, tag="ps")
                for _ in range(warmup):
                    nc.tensor.matmul(
                        wps[:], lhsT=zt[:, :128], rhs=zt[:], start=True, stop=True
                    )

            # DMA issue order == FIFO delivery order.
            wvs = []
            wv0 = cpool.tile([128, KO, 128], BF16, tag="wv0")
            nc.sync.dma_start(wv0[:], wv_d[0])
            wvs.append(wv0)

            xin = []
            xt0 = xpool.tile([128, KO, TBLK], BF16, tag="xin")
            nc.sync.dma_start(xt0[:], xt_d[0])
            xin.append(xt0)

            wv1 = cpool.tile([128, KO, 128], BF16, tag="wv1")
            nc.sync.dma_start(wv1[:], wv_d[1])
            wvs.append(wv1)
            bias_sb = cpool.tile([128, NH], F32)
            nc.sync.dma_start(bias_sb[:], bias_d[:])

            for tb in range(1, TB):
                xt = xpool.tile([128, KO, TBLK], BF16, tag="xin")
                nc.sync.dma_start(xt[:], xt_d[tb])
                xin.append(xt)

            for tb in range(TB):
                for nh in range(NH):
                    ps = ppool.tile([128, TBLK], F32, tag="ps")
                    for k in range(KO):
                        nc.tensor.matmul(
                            ps[:],
                            lhsT=wvs[nh][:, k, :],
                            rhs=xin[tb][:, k, :],
                            start=(k == 0),
                            stop=(k == KO - 1),
                        )
                    ot = opool.tile([128, TBLK], F32, tag="ot")
                    nc.vector.tensor_add(
                        ot[:],
                        ps[:],
                        bias_sb[:, nh, None].to_broadcast([128, TBLK]),
                    )
                    nc.sync.dma_start(out_d[nh, tb], ot[:])
    nc.compile()
    return nc


def _get_nc():
    # 14 warm-up matmuls measured best (ends just as the first x block
    # lands; fewer lets the HAM window reset, more delays real matmuls)
    warmup = 14
    key = ("nc3", warmup)
    if key not in _CACHE:
        _CACHE[key] = _build(warmup)
    return _CACHE[key]


def _to_bf16(a):
    import ml_dtypes

    return a.astype(ml_dtypes.bfloat16)


def _prep_inputs(x, Wkv, bkv):
    x = np.asarray(x, dtype=np.float32).reshape(TOK, E)
    Wkv = np.asarray(Wkv, dtype=np.float32)
    bkv = np.asarray(bkv, dtype=np.float32)

    xb = _to_bf16(x)
    # (core, tb, t, ko, p) -> (core, tb, p, ko, t)
    xt = xb.reshape(NCORES, TB, TBLK, KO, 128).transpose(0, 1, 4, 3, 2)
    xt = np.ascontiguousarray(xt)

    # v-columns of the kv projection: Wkv reshaped (E, G, 2, D), kv index 1.
    wv = Wkv.reshape(E, G, 2, D)[:, :, 1, :].reshape(E, NV)  # (2048, 256)
    # (ko, p, nh, n) -> (nh, p, ko, n)
    wv_dev = np.ascontiguousarray(
        _to_bf16(wv).reshape(KO, 128, NH, 128).transpose(2, 1, 0, 3)
    )
    bv = bkv.reshape(G, 2, D)[:, 1, :].reshape(NV)  # (256,)
    bias_dev = np.ascontiguousarray(bv.reshape(NH, 128).T).astype(np.float32)
    return xt, wv_dev, bias_dev


def kernel(x, Wq, bq, Wkv, bkv):
    global LAST_RESULTS
    from concourse.bass_utils import run_bass_kernel_spmd

    nc = _get_nc()
    xt, wv_dev, bias_dev = _prep_inputs(x, Wkv, bkv)
    in_maps = [
        {"xt": xt[c], "wv": wv_dev, "bias": bias_dev} for c in range(NCORES)
    ]
    res = run_bass_kernel_spmd(nc, in_maps, core_ids=list(range(NCORES)))
    LAST_RESULTS = res
    # (NH, TB, 128n, TBLKt) -> (TB, t, NH, n) -> (TPC, NV)
    y = np.stack(
        [
            res.results[c]["out"].transpose(1, 3, 0, 2).reshape(TPC, NV)
            for c in range(NCORES)
        ]
    ).reshape(TOK, NV)
    out = np.broadcast_to(
        y.reshape(TOK, G, 1, D), (TOK, G, HPG, D)
    ).reshape(B, L, E)
    return np.ascontiguousarray(out).astype(np.float32)



# revision 4
# speedup vs baseline: 1.0173x; 1.0173x over previous
"""TRN2 Bass kernel for nn_GQA_22436909154699 — optimized v4.

Reference math: softmax over a size-1 axis is identically 1.0, so
    out[b,l,g,h,:] = v[b,l,g,:]          (v = v-half of x @ Wkv + bkv)
The q projection (x @ Wq) never affects the output.  The kernel computes
    y = x @ Wv + bv                      (K=2048, N=256)
data-parallel over tokens across 8 NeuronCores (2048 tokens each).

v4 changes vs v3 (trace-driven):
  - measured fixed framework overhead is ~14us (NEFF preamble + ~8us
    post-kernel semaphore-sweep tail); the optimizable part is the
    [first-DMA .. last-output-landed] span.  v3 lost ~6us waiting for
    the whole first 1MB x block + 512KB weight DMA before the first
    matmul, and ~3us writing fp32 outputs on the same FIFO ring as the
    input stream.
  - x streams in 32 chunks of 256KB on the Sync HWDGE ring; weights +
    bias + outputs ride the Scalar (ACT) HWDGE ring so they never stall
    the x FIFO.  Weight halves are interleaved so the first matmul only
    needs 256KB x + 512KB wv.
  - per-chunk nh-interleave: each 2-ktile x chunk immediately feeds both
    column-half accumulation groups, so the PE trails the DMA stream by
    one chunk and the post-stream tail is ~1us.
  - N=512 matmuls (one full PSUM bank per group), TB=4 token blocks.
  - bf16 outputs (host upcasts): halves output DMA bytes; rounding adds
    ~0.2% error vs the 2e-2 budget.
"""

import numpy as np

# Problem constants (hardcoded; harness runs kernel.py standalone).
B, L, E = 4, 4096, 2048
G, HPG, D = 4, 8, 64
NV = G * D  # 256 v-columns
NH = NV // 128  # 2 column halves (PE stationary is 128 wide)
NCORES = 8
TOK = B * L  # 16384 tokens
TPC = TOK // NCORES  # 2048 tokens per core
TBLK = 512  # tokens per matmul rhs / PSUM group
TB = TPC // TBLK  # 4 token blocks per core
KO = E // 128  # 16 contraction tiles
XCH = 8  # x chunks per block (2 k-tiles / 256 KB each)
KPC = KO // XCH  # k-tiles per chunk = 2
WH = 2  # weight k-halves per nh (256 KB each)

_CACHE: dict = {}
LAST_RESULTS = None


def _build(warmup: int):
    import concourse.bacc as bacc
    import concourse.mybir as mybir
    import concourse.tile as tile

    F32 = mybir.dt.float32
    BF16 = mybir.dt.bfloat16

    nc = bacc.Bacc(
        "TRN2", target_bir_lowering=False, debug=False, num_devices=NCORES
    )
    xt_d = nc.dram_tensor(
        "xt", [TB, XCH, 128, KPC, TBLK], BF16, kind="ExternalInput"
    )
    wv_d = nc.dram_tensor(
        "wv", [NH, WH, 128, KO // WH, 128], BF16, kind="ExternalInput"
    )
    bias_d = nc.dram_tensor("bias", [128, NH], F32, kind="ExternalInput")
    out_d = nc.dram_tensor("out", [NH, TB, 128, TBLK], BF16, kind="ExternalOutput")

    with tile.TileContext(nc) as tc:
        with (
            tc.tile_pool(name="const", bufs=1) as cpool,
            tc.tile_pool(name="xin", bufs=TB) as xpool,
            tc.tile_pool(name="obuf", bufs=4) as opool,
            tc.tile_pool(name="ps", bufs=8, space="PSUM") as ppool,
        ):
            # PE warm-up on zeros while the first DMAs land (HAM un-throttle).
            if warmup:
                zt = cpool.tile([128, TBLK], BF16)
                nc.vector.memset(zt[:], 0.0)
                wps = ppool.tile([128, TBLK], F32, tag="ps")
                for _ in range(warmup):
                    nc.tensor.matmul(
                        wps[:], lhsT=zt[:, :128], rhs=zt[:], start=True, stop=True
                    )

            # Weights + bias on the ACT HWDGE ring: halves interleaved so
            # (wv0 k0-7, wv1 k0-7) land before (k8-15) halves.
            wvs = []
            for nh in range(NH):
                wvs.append(
                    cpool.tile(
                        [128, KO, 128], BF16, tag=f"wv{nh}", name=f"wv{nh}"
                    )
                )
            for h in range(WH):
                for nh in range(NH):
                    nc.scalar.dma_start(
                        wvs[nh][:, h * (KO // WH) : (h + 1) * (KO // WH), :],
                        wv_d[nh, h],
                    )
            bias_sb = cpool.tile([128, NH], F32)
            nc.scalar.dma_start(bias_sb[:], bias_d[:])

            # x stream: 32 x 256KB chunks, FIFO on the Sync HWDGE ring.
            xin = []
            for tb in range(TB):
                xt = xpool.tile([128, KO, TBLK], BF16, tag="xin")
                for c in range(XCH):
                    nc.sync.dma_start(
                        xt[:, c * KPC : (c + 1) * KPC, :], xt_d[tb, c]
                    )
                xin.append(xt)

            for tb in range(TB):
                pss = [
                    ppool.tile([128, TBLK], F32, tag="ps", name=f"ps{tb}_{i}")
                    for i in range(NH)
                ]
                for c in range(XCH):
                    for nh in range(NH):
                        for kk in range(KPC):
                            k = c * KPC + kk
                            nc.tensor.matmul(
                                pss[nh][:],
                                lhsT=wvs[nh][:, k, :],
                                rhs=xin[tb][:, k, :],
                                start=(k == 0),
                                stop=(k == KO - 1),
                            )
                for nh in range(NH):
                    ot = opool.tile([128, TBLK], BF16, tag="ot")
                    nc.vector.tensor_add(
                        ot[:],
                        pss[nh][:],
                        bias_sb[:, nh, None].to_broadcast([128, TBLK]),
                    )
                    nc.scalar.dma_start(out_d[nh, tb], ot[:])
    nc.compile()
    return nc


def _get_nc():
    warmup = 6
    key = ("nc4", warmup)
    if key not in _CACHE:
        _CACHE[key] = _build(warmup)
    return _CACHE[key]


def _to_bf16(a):
    import ml_dtypes

    return a.astype(ml_dtypes.bfloat16)


def _prep_inputs(x, Wkv, bkv):
    x = np.asarray(x, dtype=np.float32).reshape(TOK, E)
    Wkv = np.asarray(Wkv, dtype=np.float32)
    bkv = np.asarray(bkv, dtype=np.float32)

    xb = _to_bf16(x)
    # (core, tb, t, c, kk, p) -> (core, tb, c, p, kk, t)
    xt = xb.reshape(NCORES, TB, TBLK, XCH, KPC, 128).transpose(0, 1, 3, 5, 4, 2)
    xt = np.ascontiguousarray(xt)

    # v-columns of the kv projection: Wkv reshaped (E, G, 2, D), kv index 1.
    wv = Wkv.reshape(E, G, 2, D)[:, :, 1, :].reshape(E, NV)  # (2048, 256)
    # e = (h*8+kh)*128 + p, col = nh*128 + n:
    # (h, kh, p, nh, n) -> (nh, h, p, kh, n)
    wv_dev = np.ascontiguousarray(
        _to_bf16(wv).reshape(WH, KO // WH, 128, NH, 128).transpose(3, 0, 2, 1, 4)
    )
    bv = bkv.reshape(G, 2, D)[:, 1, :].reshape(NV)  # (256,)
    bias_dev = np.ascontiguousarray(bv.reshape(NH, 128).T).astype(np.float32)
    return xt, wv_dev, bias_dev


def kernel(x, Wq, bq, Wkv, bkv):
    global LAST_RESULTS
    from concourse.bass_utils import run_bass_kernel_spmd

    nc = _get_nc()
    xt, wv_dev, bias_dev = _prep_inputs(x, Wkv, bkv)
    in_maps = [
        {"xt": xt[c], "wv": wv_dev, "bias": bias_dev} for c in range(NCORES)
    ]
    res = run_bass_kernel_spmd(nc, in_maps, core_ids=list(range(NCORES)))
    LAST_RESULTS = res
    # (NH, TB, 128n, TBLKt) -> (TB, t, NH, n) -> (TPC, NV)
    y = np.stack(
        [
            np.asarray(res.results[c]["out"])
            .astype(np.float32)
            .transpose(1, 3, 0, 2)
            .reshape(TPC, NV)
            for c in range(NCORES)
        ]
    ).reshape(TOK, NV)
    out = np.broadcast_to(
        y.reshape(TOK, G, 1, D), (TOK, G, HPG, D)
    ).reshape(B, L, E)
    return np.ascontiguousarray(out).astype(np.float32)


# revision 5
# speedup vs baseline: 1.0401x; 1.0225x over previous
"""TRN2 Bass kernel for nn_GQA_22436909154699 — optimized v5.

Reference math: softmax over a size-1 axis is identically 1.0, so
    out[b,l,g,h,:] = v[b,l,g,:]          (v = v-half of x @ Wkv + bkv)
The q projection (x @ Wq) never affects the output.  The kernel computes
    y = x @ Wv + bv                      (K=2048, N=256)
data-parallel over tokens across 8 NeuronCores (2048 tokens each).

Measured fixed framework cost is ~10us inside the counted window (~1us
preamble + ~8.6us end-of-NEFF semaphore sweep), so the optimizable span
is [first DMA .. last output landed]:
  - x streams as 16 x 512KB chunks on the Sync HWDGE ring (512KB keeps
    the SDMA engines near line rate; 256KB chunks measured only 78%).
  - weights (k-halves interleaved wv0a,wv1a,wv0b,wv1b) + bias ride the
    Scalar/ACT ring in parallel, so the first matmul only gates on
    ~0.5MB of weights + one x chunk.
  - per-chunk nh-interleave: each 4-ktile chunk feeds both column-half
    PSUM groups immediately; the PE trails the stream by one chunk and
    the post-stream tail is ~1.7us.
  - bf16 outputs, block outputs split across both rings (nh0 on Sync,
    nh1 on ACT) so the final adds + stores overlap.
  - 5 PE warm-ups on zeros cover the HAM un-throttle before real MMs.
"""

import numpy as np

# Problem constants (hardcoded; harness runs kernel.py standalone).
B, L, E = 4, 4096, 2048
G, HPG, D = 4, 8, 64
NV = G * D  # 256 v-columns
NH = NV // 128  # 2 column halves (PE stationary is 128 wide)
NCORES = 8
TOK = B * L  # 16384 tokens
TPC = TOK // NCORES  # 2048 tokens per core
TBLK = 512  # tokens per matmul rhs / PSUM group
TB = TPC // TBLK  # 4 token blocks per core
KO = E // 128  # 16 contraction tiles
XCH = 4  # x chunks per block (512 KB each)
KPC = KO // XCH  # k-tiles per chunk = 4
WH = 2  # weight k-halves per nh (256 KB each)

_CACHE: dict = {}
LAST_RESULTS = None


def _build(warmup: int):
    import concourse.bacc as bacc
    import concourse.mybir as mybir
    import concourse.tile as tile

    F32 = mybir.dt.float32
    BF16 = mybir.dt.bfloat16

    nc = bacc.Bacc(
        "TRN2", target_bir_lowering=False, debug=False, num_devices=NCORES
    )
    xt_d = nc.dram_tensor(
        "xt", [TB, XCH, 128, KPC, TBLK], BF16, kind="ExternalInput"
    )
    wv_d = nc.dram_tensor(
        "wv", [NH, WH, 128, KO // WH, 128], BF16, kind="ExternalInput"
    )
    bias_d = nc.dram_tensor("bias", [128, NH], F32, kind="ExternalInput")
    out_d = nc.dram_tensor("out", [NH, TB, 128, TBLK], BF16, kind="ExternalOutput")

    with tile.TileContext(nc) as tc:
        with (
            tc.tile_pool(name="const", bufs=1) as cpool,
            tc.tile_pool(name="xin", bufs=TB) as xpool,
            tc.tile_pool(name="obuf", bufs=4) as opool,
            tc.tile_pool(name="ps", bufs=8, space="PSUM") as ppool,
        ):
            # PE warm-up on zeros while the first DMAs land (HAM un-throttle).
            if warmup:
                zt = cpool.tile([128, TBLK], BF16)
                nc.vector.memset(zt[:], 0.0)
                wps = ppool.tile([128, TBLK], F32, tag="ps")
                for _ in range(warmup):
                    nc.tensor.matmul(
                        wps[:], lhsT=zt[:, :128], rhs=zt[:], start=True, stop=True
                    )

            # Weights + bias on the ACT HWDGE ring, k-halves interleaved so
            # (wv0 k0-7, wv1 k0-7) land before the k8-15 halves.
            wvs = []
            for nh in range(NH):
                wvs.append(
                    cpool.tile(
                        [128, KO, 128], BF16, tag=f"wv{nh}", name=f"wv{nh}"
                    )
                )
            for h in range(WH):
                for nh in range(NH):
                    nc.scalar.dma_start(
                        wvs[nh][:, h * (KO // WH) : (h + 1) * (KO // WH), :],
                        wv_d[nh, h],
                    )
            bias_sb = cpool.tile([128, NH], F32)
            nc.scalar.dma_start(bias_sb[:], bias_d[:])

            # x stream: 16 x 512KB chunks, FIFO on the Sync HWDGE ring.
            xin = []
            for tb in range(TB):
                xt = xpool.tile([128, KO, TBLK], BF16, tag="xin")
                for c in range(XCH):
                    nc.sync.dma_start(
                        xt[:, c * KPC : (c + 1) * KPC, :], xt_d[tb, c]
                    )
                xin.append(xt)

            for tb in range(TB):
                pss = [
                    ppool.tile([128, TBLK], F32, tag="ps", name=f"ps{tb}_{i}")
                    for i in range(NH)
                ]
                for c in range(XCH):
                    for nh in range(NH):
                        for kk in range(KPC):
                            k = c * KPC + kk
                            nc.tensor.matmul(
                                pss[nh][:],
                                lhsT=wvs[nh][:, k, :],
                                rhs=xin[tb][:, k, :],
                                start=(k == 0),
                                stop=(k == KO - 1),
                            )
                for nh in range(NH):
                    ot = opool.tile([128, TBLK], BF16, tag="ot", name=f"ot{tb}_{nh}")
                    nc.vector.tensor_add(
                        ot[:],
                        pss[nh][:],
                        bias_sb[:, nh, None].to_broadcast([128, TBLK]),
                    )
                    eng = nc.sync if nh == 0 else nc.scalar
                    eng.dma_start(out_d[nh, tb], ot[:])
    nc.compile()
    return nc


def _get_nc():
    warmup = 5
    key = ("nc5", warmup)
    if key not in _CACHE:
        _CACHE[key] = _build(warmup)
    return _CACHE[key]


def _to_bf16(a):
    import ml_dtypes

    return a.astype(ml_dtypes.bfloat16)


def _prep_inputs(x, Wkv, bkv):
    x = np.asarray(x, dtype=np.float32).reshape(TOK, E)
    Wkv = np.asarray(Wkv, dtype=np.float32)
    bkv = np.asarray(bkv, dtype=np.float32)

    xb = _to_bf16(x)
    # (core, tb, t, c, kk, p) -> (core, tb, c, p, kk, t)
    xt = xb.reshape(NCORES, TB, TBLK, XCH, KPC, 128).transpose(0, 1, 3, 5, 4, 2)
    xt = np.ascontiguousarray(xt)

    # v-columns of the kv projection: Wkv reshaped (E, G, 2, D), kv index 1.
    wv = Wkv.reshape(E, G, 2, D)[:, :, 1, :].reshape(E, NV)  # (2048, 256)
    # e = (h*8+kh)*128 + p, col = nh*128 + n:
    # (h, kh, p, nh, n) -> (nh, h, p, kh, n)
    wv_dev = np.ascontiguousarray(
        _to_bf16(wv).reshape(WH, KO // WH, 128, NH, 128).transpose(3, 0, 2, 1, 4)
    )
    bv = bkv.reshape(G, 2, D)[:, 1, :].reshape(NV)  # (256,)
    bias_dev = np.ascontiguousarray(bv.reshape(NH, 128).T).astype(np.float32)
    return xt, wv_dev, bias_dev


def kernel(x, Wq, bq, Wkv, bkv):
    global LAST_RESULTS
    from concourse.bass_utils import run_bass_kernel_spmd

    nc = _get_nc()
    xt, wv_dev, bias_dev = _prep_inputs(x, Wkv, bkv)
    in_maps = [
        {"xt": xt[c], "wv": wv_dev, "bias": bias_dev} for c in range(NCORES)
    ]
    res = run_bass_kernel_spmd(nc, in_maps, core_ids=list(range(NCORES)))
    LAST_RESULTS = res
    # (NH, TB, 128n, TBLKt) -> (TB, t, NH, n) -> (TPC, NV)
    y = np.stack(
        [
            np.asarray(res.results[c]["out"])
            .astype(np.float32)
            .transpose(1, 3, 0, 2)
            .reshape(TPC, NV)
            for c in range(NCORES)
        ]
    ).reshape(TOK, NV)
    out = np.broadcast_to(
        y.reshape(TOK, G, 1, D), (TOK, G, HPG, D)
    ).reshape(B, L, E)
    return np.ascontiguousarray(out).astype(np.float32)
